# revision 1
# baseline (speedup 1.0000x reference)
"""Trainium2 Bass kernel for DeepKernelRegressionModel.

Math (per core, X sharded by rows across 8 cores):
  Xf = MLP(X), Yf = MLP(Y)                        (3-layer relu MLP, H=32)
  K[i,m] = exp(-|Xf_i - Yf_m|^2 / 2)
         = exp(Xf_i . Yf_m - |Xf_i|^2/2 - |Yf_m|^2/2)
  out = (K @ Y_target) / (K @ 1)

Everything is fused: the exponent is produced by ONE tensor-engine matmul
with an augmented contraction dim (K=34):
  lhsT rows 0-31 = Yf^T, row 32 = 1,       row 33 = -|Yf|^2/2
  rhs  rows 0-31 = Xf^T, row 32 = -|Xf|^2/2, row 33 = 1
in the transposed orientation G'[m, i], so that the second matmul
  acc[t, i] += Z_chunk^T @ exp(G')      with Z = [Y_target, 1]
contracts over m (the partition dim) with no transposes of the big
exp matrix. A final tiny transpose + reciprocal produces out[i, t].

The MLPs run in the transposed orientation (features on partitions) with
4-way tile_position stacking so relu ops use all 128 partitions.
"""

import os
import numpy as np
from contextlib import ExitStack

import concourse.bass as bass
import concourse.tile as tile
from concourse import bacc, mybir

FP = mybir.dt.float32
FPR = mybir.dt.float32r
AF = mybir.ActivationFunctionType

D, H, T = 64, 32, 8
TZ = T + 1  # Y_target columns + ones column
ZP = 32     # Z padded to 32 cols so mm2 fully writes its PSUM stripes
N_CORES = 8


def _split_matmul_waits(nc):
    """Walrus's S3_LW lowering for self-loading (4-byte) matmuls supports only
    one sync-wait command. Move multi-waits onto a PE sequencer NoOp placed
    right before the matmul — the in-order NX applies them to the stream."""
    import bass_rust

    k = 0
    for fn in nc.m.functions:
        for blk in fn.blocks:
            out = []
            for inst in blk.instructions:
                si = inst.sync_info
                if (type(inst).__name__ == "InstMatmult" and si is not None
                        and si.on_wait and len(si.on_wait) >= 2):
                    waits = list(si.on_wait)
                    for w in waits[:-1]:
                        nop = mybir.InstNoOp(name=f"I-mmwait-{k}", ins=[],
                                             outs=[])
                        k += 1
                        nop.engine = inst.engine
                        nop.sync_info = bass_rust.SyncInfo(
                            on_wait=[w], on_update=[])
                        out.append(nop)
                    inst.sync_info = bass_rust.SyncInfo(
                        on_wait=[waits[-1]], on_update=list(si.on_update))
                out.append(inst)
            blk.instructions = out


def build_nc(n_sh, m_total, use_f32r=True, exp_group=3, split_waits=True):
    """Build the Bass program for one core (SPMD: same program, all cores).

    n_sh: rows of X handled by this core. m_total: rows of Y (full).
    """
    assert n_sh % 512 == 0 and m_total % 2048 == 0
    MT = m_total // 128       # number of 128-row m-tiles
    NCH = m_total // 512      # number of 512-wide m-chunks (MLP)
    XG = n_sh // 4            # X stacked-chunk width
    IC = n_sh // 512          # i-chunks
    ICW = 512

    def r(ap):
        return ap.bitcast(FPR) if use_f32r else ap

    nc = bacc.Bacc("TRN2", target_bir_lowering=False, debug=False,
                   num_devices=N_CORES)

    Xd = nc.dram_tensor("X", [n_sh, D], FP, kind="ExternalInput").ap()
    Yd = nc.dram_tensor("Y", [m_total, D], FP, kind="ExternalInput").ap()
    Zd = nc.dram_tensor("Zm", [m_total, ZP], FP, kind="ExternalInput").ap()
    W1d = nc.dram_tensor("W1", [D, H], FP, kind="ExternalInput").ap()
    W2d = nc.dram_tensor("W2", [H, H], FP, kind="ExternalInput").ap()
    W3d = nc.dram_tensor("W3", [H, H], FP, kind="ExternalInput").ap()
    Bd = nc.dram_tensor("Bs", [128, 3], FP, kind="ExternalInput").ap()
    Id = nc.dram_tensor("ident", [128, 128], FP, kind="ExternalInput").ap()
    NHd = nc.dram_tensor("neghalf", [128, 32], FP, kind="ExternalInput").ap()
    ORd = nc.dram_tensor("onesrow", [1, m_total], FP, kind="ExternalInput").ap()
    OUTd = nc.dram_tensor("out", [n_sh, T], FP, kind="ExternalOutput").ap()

    with tile.TileContext(nc) as tc, ExitStack() as ctx:
        const = ctx.enter_context(tc.tile_pool(name="const", bufs=1))
        big = ctx.enter_context(tc.tile_pool(name="big", bufs=1))
        scr = ctx.enter_context(tc.tile_pool(name="scr", bufs=1))

        w1s = const.tile([D, H], FP)
        nc.sync.dma_start(w1s[:], W1d[:])
        w2s = const.tile([128, H], FP)
        w3s = const.tile([128, H], FP)
        for g in range(4):
            nc.sync.dma_start(w2s[32 * g:32 * g + 32, :], W2d[:])
            nc.sync.dma_start(w3s[32 * g:32 * g + 32, :], W3d[:])
        bs = const.tile([128, 3], FP)
        nc.sync.dma_start(bs[:], Bd[:])
        ident = const.tile([128, 128], FP)
        nc.sync.dma_start(ident[:], Id[:])
        nh = const.tile([128, 32], FP)
        nc.sync.dma_start(nh[:], NHd[:])
        zt = const.tile([128, MT * ZP], FP)
        nc.sync.dma_start(
            r(zt.rearrange("p (t c) -> p t c", c=ZP)),
            r(Zd.rearrange("(t p) c -> p t c", p=128)),
        )

        # persistent big tensors
        yT = big.tile([D, m_total], FP)      # Y^T
        xT = big.tile([D, n_sh], FP)         # X^T
        yft = big.tile([128, m_total], FP)   # rows 0-33 aug A, 64-97 aug B
        xft = big.tile([128, n_sh], FP)

        # ---------------- phase A: transposes (PE) ----------------
        with (
            tc.tile_pool(name="tp_psum", bufs=2, space="PSUM") as tpp,
            tc.tile_pool(name="ytile", bufs=4) as ytp,
        ):
            n_ych = (MT + 7) // 8
            for c in range(n_ych):
                ts = list(range(8 * c, min(8 * c + 8, MT)))
                tp = tpp.tile([D, 128 * len(ts)], FP, tag="tp")
                for k, mt in enumerate(ts):
                    ytile = ytp.tile([128, D], FP, tag="yt")
                    nc.sync.dma_start(ytile[:], Yd[128 * mt:128 * mt + 128, :])
                    nc.tensor.transpose(tp[:, 128 * k:128 * k + 128],
                                        ytile[:], ident[:])
                nc.vector.tensor_copy(
                    yT[:, 1024 * c:1024 * c + 128 * len(ts)], tp[:])
            n_xch = (n_sh // 128 + 7) // 8
            for c in range(n_xch):
                ts = list(range(8 * c, min(8 * c + 8, n_sh // 128)))
                tp = tpp.tile([D, 128 * len(ts)], FP, tag="tp")
                for k, mt in enumerate(ts):
                    xtile = ytp.tile([128, D], FP, tag="yt")
                    nc.sync.dma_start(xtile[:], Xd[128 * mt:128 * mt + 128, :])
                    nc.tensor.transpose(tp[:, 128 * k:128 * k + 128],
                                        xtile[:], ident[:])
                nc.vector.tensor_copy(
                    xT[:, 1024 * c:1024 * c + 128 * len(ts)], tp[:])

        # ---------------- phase B: Y MLP (stacked 4x) ----------------
        # chunk ch (512 m's) -> partition group cg = ch%4, col chunk cc = ch//4
        CCY = NCH // 4
        yfp = ctx.enter_context(tc.tile_pool(name="yf_pool", bufs=1))
        with (
            tc.tile_pool(name="mlp_psum", bufs=2, space="PSUM") as mpp,
            tc.tile_pool(name="acts", bufs=2) as actp,
        ):
            h1p = mpp.tile([128, 512 * CCY], FP, tag="hp")
            for ch in range(NCH):
                cg, cc = ch % 4, ch // 4
                nc.tensor.matmul(h1p[32 * cg:32 * cg + 32, 512 * cc:512 * cc + 512],
                                 lhsT=w1s[:], rhs=yT[:, 512 * ch:512 * ch + 512],
                                 start=True, stop=True,
                                 skip_group_check=True,
                                 tile_position=(0, 32 * cg))
            h1s = actp.tile([128, 512 * CCY], FP, tag="hs")
            for cc in range(CCY):
                nc.scalar.activation(h1s[:, 512 * cc:512 * cc + 512],
                                     h1p[:, 512 * cc:512 * cc + 512],
                                     AF.Relu, bias=bs[:, 0:1])
            h2p = mpp.tile([128, 512 * CCY], FP, tag="hp")
            for ch in range(NCH):
                cg, cc = ch % 4, ch // 4
                nc.tensor.matmul(h2p[32 * cg:32 * cg + 32, 512 * cc:512 * cc + 512],
                                 tile_position=(32 * cg, 32 * cg),
                                 lhsT=w2s[32 * cg:32 * cg + 32, :],
                                 rhs=h1s[32 * cg:32 * cg + 32, 512 * cc:512 * cc + 512],
                                 start=True, stop=True,
                                 skip_group_check=True)
            h2s = actp.tile([128, 512 * CCY], FP, tag="hs")
            for cc in range(CCY):
                nc.scalar.activation(h2s[:, 512 * cc:512 * cc + 512],
                                     h2p[:, 512 * cc:512 * cc + 512],
                                     AF.Relu, bias=bs[:, 1:2])
            h3p = mpp.tile([128, 512 * CCY], FP, tag="hp")
            for ch in range(NCH):
                cg, cc = ch % 4, ch // 4
                nc.tensor.matmul(h3p[32 * cg:32 * cg + 32, 512 * cc:512 * cc + 512],
                                 tile_position=(32 * cg, 32 * cg),
                                 lhsT=w3s[32 * cg:32 * cg + 32, :],
                                 rhs=h2s[32 * cg:32 * cg + 32, 512 * cc:512 * cc + 512],
                                 start=True, stop=True,
                                 skip_group_check=True)
            yfs = yfp.tile([128, 512 * CCY], FP, tag="yfs")
            sqy = yfp.tile([128, 512 * CCY], FP, tag="sqy")
            for cc in range(CCY):
                nc.scalar.activation(r(yfs[:, 512 * cc:512 * cc + 512]),
                                     h3p[:, 512 * cc:512 * cc + 512],
                                     AF.Relu, bias=bs[:, 2:3])
                nc.vector.tensor_mul(sqy[:, 512 * cc:512 * cc + 512],
                                     yfs[:, 512 * cc:512 * cc + 512],
                                     yfs[:, 512 * cc:512 * cc + 512])
            # assemble yft rows 0-31 (flat layout)
            for ch in range(NCH):
                cg, cc = ch % 4, ch // 4
                nc.sync.dma_start(r(yft[0:32, 512 * ch:512 * ch + 512]),
                                  r(yfs[32 * cg:32 * cg + 32, 512 * cc:512 * cc + 512]))
            nc.sync.dma_start(r(yft[32:33, :]), r(ORd[:]))  # ones row

        # ---------------- phase C: norms + X MLP ----------------
        with (
            tc.tile_pool(name="ynp", bufs=2, space="PSUM") as ynpp,
            tc.tile_pool(name="xnp", bufs=1, space="PSUM") as xnpp,
        ):
            # ynorm row: -|Yf_m|^2/2 for every m, laid out on partitions
            # {0,32,64,96} x 1024 cols per psum tile (one tile per 4096 m)
            n_yn = (NCH + 7) // 8
            for a in range(n_yn):
                chs = list(range(8 * a, min(8 * a + 8, NCH)))
                ynp = ynpp.tile([128, 1024], FP, tag="ynp")
                for chl, ch in enumerate(chs):
                    cg, cc = ch % 4, ch // 4
                    prow, pcol = 32 * (chl // 2), 512 * (chl % 2)
                    nc.tensor.matmul(ynp[prow:prow + 32, pcol:pcol + 512],
                                     tile_position=(32 * cg, prow),
                                     lhsT=nh[32 * cg:32 * cg + 32, :],
                                     rhs=sqy[32 * cg:32 * cg + 32,
                                             512 * cc:512 * cc + 512],
                                     start=True, stop=True,
                                     skip_group_check=True)
                yns = scr.tile([128, 1024], FP, tag="yns")
                nrow = 32 * ((len(chs) + 1) // 2)
                nc.vector.tensor_copy(r(yns[0:nrow, :]), ynp[0:nrow, :])
                for k in range(len(chs) // 2):
                    nc.sync.dma_start(
                        r(yft[33:34, 4096 * a + 1024 * k:4096 * a + 1024 * k + 1024]),
                        r(yns[32 * k:32 * k + 1, :]))
            # duplicate augmented block to partitions 64-97 (row group B)
            for sg in range(4):
                seg = m_total // 4
                nc.sync.dma_start(r(yft[64:98, seg * sg:seg * sg + seg]),
                                  r(yft[0:34, seg * sg:seg * sg + seg]))

            # ---- X MLP (4 chunks of XG cols, stacked) ----
            hx1 = xnpp.tile([128, XG], FP, tag="hx")
            for ch in range(4):
                nc.tensor.matmul(hx1[32 * ch:32 * ch + 32, :],
                                 tile_position=(0, 32 * ch),
                                 lhsT=w1s[:],
                                 rhs=xT[:, XG * ch:XG * ch + XG],
                                 start=True, stop=True,
                                 skip_group_check=True)
            hx1s = scr.tile([128, XG], FP, tag="hxs1")
            nc.scalar.activation(hx1s[:], hx1[:], AF.Relu, bias=bs[:, 0:1])
            hx2 = xnpp.tile([128, XG], FP, tag="hx")
            for ch in range(4):
                nc.tensor.matmul(hx2[32 * ch:32 * ch + 32, :],
                                 tile_position=(32 * ch, 32 * ch),
                                 lhsT=w2s[32 * ch:32 * ch + 32, :],
                                 rhs=hx1s[32 * ch:32 * ch + 32, :],
                                 start=True, stop=True,
                                 skip_group_check=True)
            hx2s = scr.tile([128, XG], FP, tag="hxs2")
            nc.scalar.activation(hx2s[:], hx2[:], AF.Relu, bias=bs[:, 1:2])
            hx3 = xnpp.tile([128, XG], FP, tag="hx")
            for ch in range(4):
                nc.tensor.matmul(hx3[32 * ch:32 * ch + 32, :],
                                 tile_position=(32 * ch, 32 * ch),
                                 lhsT=w3s[32 * ch:32 * ch + 32, :],
                                 rhs=hx2s[32 * ch:32 * ch + 32, :],
                                 start=True, stop=True,
                                 skip_group_check=True)
            xfs = scr.tile([128, XG], FP, tag="xfs")
            nc.scalar.activation(r(xfs[:]), hx3[:], AF.Relu, bias=bs[:, 2:3])
            sqx = scr.tile([128, XG], FP, tag="sqx")
            nc.vector.tensor_mul(sqx[:], xfs[:], xfs[:])
            for ch in range(4):
                nc.sync.dma_start(r(xft[0:32, XG * ch:XG * ch + XG]),
                                  r(xfs[32 * ch:32 * ch + 32, :]))
            nc.sync.dma_start(r(xft[33:34, :]), r(ORd[0:1, 0:n_sh]))  # ones row
            xnp = xnpp.tile([128, XG], FP, tag="xnp")
            for ch in range(4):
                nc.tensor.matmul(xnp[32 * ch:32 * ch + 32, :],
                                 tile_position=(32 * ch, 32 * ch),
                                 lhsT=nh[32 * ch:32 * ch + 32, :],
                                 rhs=sqx[32 * ch:32 * ch + 32, :],
                                 start=True, stop=True,
                                 skip_group_check=True)
            xns = scr.tile([128, XG], FP, tag="xns")
            nc.vector.tensor_copy(r(xns[:]), xnp[:])
            for ch in range(4):
                nc.sync.dma_start(r(xft[32:33, XG * ch:XG * ch + XG]),
                                  r(xns[32 * ch:32 * ch + 1, :]))
            nc.sync.dma_start(r(xft[64:98, :]), r(xft[0:34, :]))

        # ---------------- main loop ----------------
        groups = []
        mt = 0
        while mt < MT:
            groups.append(list(range(mt, min(mt + exp_group, MT))))
            mt += exp_group

        with (
            tc.tile_pool(name="gbuf", bufs=2, space="PSUM") as gpool,
            tc.tile_pool(name="accp", bufs=2, space="PSUM") as apool,
            tc.tile_pool(name="ebuf", bufs=3) as epool,
            tc.tile_pool(name="fin", bufs=2) as finp,
        ):
            for ic in range(IC):
                acc = apool.tile([128, ICW], FP, tag="acc")
                for grp in groups:
                    gp = gpool.tile([128, 512 * exp_group], FP, tag="g")
                    for t, mt in enumerate(grp):
                        rg = 64 * (mt % 2)
                        nc.tensor.matmul(
                            gp[:, 512 * t:512 * t + 512],
                            tile_position=(rg, 0),
                            lhsT=r(yft[rg:rg + 34, 128 * mt:128 * mt + 128]),
                            rhs=r(xft[rg:rg + 34, ICW * ic:ICW * ic + ICW]),
                            start=True, stop=True)
                    eb = epool.tile([128, 512 * exp_group], FP, tag="e")
                    w = 512 * len(grp)
                    nc.scalar.activation(r(eb[:, :w]), gp[:, :w], AF.Exp)
                    for t, mt in enumerate(grp):
                        nc.tensor.matmul(
                            acc[0:32, :],
                            tile_position=(0, 0),
                            lhsT=r(zt[:, ZP * mt:ZP * mt + ZP]),
                            rhs=r(eb[:, 512 * t:512 * t + 512]),
                            start=(mt == 0), stop=(mt == MT - 1),
                            skip_group_check=True)
                # fold 4 col-group accumulators via transpose-accumulate
                acc_s = finp.tile([32, ICW], FP, tag="accs")
                nc.vector.tensor_copy(acc_s[:], acc[0:32, :])
                ot = apool.tile([128, 128], FP, tag="acc")
                for q in range(4):
                    nc.tensor.matmul(
                        ot[:, 32 * q:32 * q + 32],
                        tile_position=(0, 0),
                        lhsT=acc_s[0:32, 128 * q:128 * q + 128],
                        rhs=ident[0:32, 0:32],
                        is_transpose=True,
                        start=(q == 0), stop=(q == 3),
                        skip_group_check=True)
                for q in range(4):
                    rec = finp.tile([128, 1], FP, tag="rec")
                    nc.vector.reciprocal(rec[:], ot[:, 32 * q + T:32 * q + T + 1])
                    res = finp.tile([128, T], FP, tag="res")
                    nc.vector.tensor_scalar_mul(res[:], ot[:, 32 * q:32 * q + T],
                                                rec[:])
                    nc.sync.dma_start(
                        OUTd[ICW * ic + 128 * q:ICW * ic + 128 * q + 128, :],
                        res[:])
    nc.compile()
    return nc


def make_in_maps(X, Y, Y_target, W1, b1, W2, b2, W3, b3, n_cores=N_CORES):
    f = lambda a: np.ascontiguousarray(np.asarray(a, dtype=np.float32))
    X, Y, Y_target = f(X), f(Y), f(Y_target)
    W1, W2, W3 = f(W1), f(W2), f(W3)
    b1, b2, b3 = f(b1), f(b2), f(b3)
    m_total = Y.shape[0]
    n_sh = X.shape[0] // n_cores
    Zm = np.zeros((m_total, 32), np.float32)
    Zm[:, :T] = Y_target
    Zm[:, T] = 1.0
    Bs = np.stack([np.tile(b1, 4), np.tile(b2, 4), np.tile(b3, 4)], axis=1)
    common = dict(
        Y=Y, Zm=Zm, W1=W1, W2=W2, W3=W3,
        Bs=np.ascontiguousarray(Bs),
        ident=np.eye(128, dtype=np.float32),
        neghalf=np.full((128, 32), -0.5, np.float32),
        onesrow=np.ones((1, m_total), np.float32),
    )
    return [dict(common, X=X[c * n_sh:(c + 1) * n_sh]) for c in range(n_cores)]


_NC_CACHE = {}


def _get_nc(n_sh, m_total):
    key = (n_sh, m_total)
    if key not in _NC_CACHE:
        use_f32r = os.environ.get("DKR_F32R", "1") == "1"
        _NC_CACHE[key] = build_nc(n_sh, m_total, use_f32r=use_f32r)
    return _NC_CACHE[key]


def kernel(X, Y, Y_target, W1, b1, W2, b2, W3, b3):
    from concourse.bass_utils import run_bass_kernel_spmd

    in_maps = make_in_maps(X, Y, Y_target, W1, b1, W2, b2, W3, b3)
    n_sh = in_maps[0]["X"].shape[0]
    nc = _get_nc(n_sh, np.asarray(Y).shape[0])
    res = run_bass_kernel_spmd(nc, in_maps, core_ids=list(range(N_CORES)))
    return np.concatenate([res.results[c]["out"] for c in range(N_CORES)], axis=0)



# revision 21
# speedup vs baseline: 817.3490x; 817.3490x over previous
"""Trainium2 Bass kernel for DeepKernelRegressionModel.

Math (per core, X sharded by rows across 8 cores):
  Xf = MLP(X), Yf = MLP(Y)                        (3-layer relu MLP, H=32)
  K[i,m] = exp(-|Xf_i - Yf_m|^2 / 2)
  out = (K @ Y_target) / (K @ 1)

Two identities remove all exponent augmentation:
  - the factor exp(-|Xf_i|^2/2) is constant per row i and cancels in the
    normalization, so it is never computed;
  - the factor exp(-|Yf_m|^2/2) is folded into the Z matrix instead of
    the exponent:  z'[m,:] = z[m,:] * exp(-|Yf_m|^2/2).
So mm1 is a pure K=32 product G'[m,i] = Yf_m . Xf_i (m on psum
partitions, 128 per tile), exp runs on ACT, and
  acc[t,i] += z'^T @ exp(G')
accumulates over all m.  A final tiny transpose + reciprocal normalizes.

Hardware constraints that shape the design (verified on neuronx-cc):
  - f32r matmuls (full-rate fp32, 1 cycle/row) are only legal with
    tile_position column 0, so every matmul keeps outputs at partition
    base 0: the MLPs run UNSTACKED on partitions 0-31 and the relus
    (DVE tensor_scalar add+max) pay free-size cost instead;
  - every f32r-matmul operand must be produced through an f32r-typed
    write (loads and relu/square outputs are bitcast);
  - GPSIMD cannot touch PSUM, so Pool only carries DMAs;
  - DMA cost is ~(per-partition bytes x 0.39ns), so X^T/Y^T are host-
    packed two 64-row bands high (W1 is duplicated to rows 64-127).

The kernel runs as one uniform per-chunk pipeline (512 Y rows each):
MLP -> relu (DVE) -> square -> norm matmul [1,512] -> 1-row PE
transposes into [p=m%128, mt] PSUM -> ACT exp -> DVE z-scale, then
immediately the main-loop groups (mm1 -> exp -> mm2) for that chunk's
four m-tiles on both i-chunks, so ACT streams exps almost from the
start and PE never waits on far-away phases.
"""

import os
import numpy as np
from contextlib import ExitStack

import concourse.bass as bass
import concourse.tile as tile
from concourse import bacc, mybir
from concourse.alu_op_type import AluOpType

FP = mybir.dt.float32
FPR = mybir.dt.float32r
AF = mybir.ActivationFunctionType

D, H, T = 64, 32, 8
ZP = 16     # Z columns per m-tile: 8 targets, 1 ones, 7 pad
N_CORES = 8

# packed-constant column layout (consts tile [128, CW])
C_W1, C_W2, C_W3, C_BS, C_ID, C_NH, C_ONE = 0, 32, 64, 96, 99, 115, 147
CW = 160


def build_nc(n_sh, m_total, use_f32r=True, exp_group=2):
    """Build the Bass program for one core (SPMD: same program, all cores).

    n_sh: rows of X handled by this core. m_total: rows of Y (full).
    """
    assert n_sh % 512 == 0 and m_total % 2048 == 0
    MT = m_total // 128       # number of 128-row m-tiles
    NCH = m_total // 512      # number of 512-wide m-chunks
    IC = n_sh // 512          # i-chunks
    ICW = 512
    GA = exp_group            # m-tiles per exp group (4 % GA == 0)
    PY = m_total // 2048      # packed Y loads
    VX = n_sh // 512          # X 512-col chunks
    XROWS = 64 * min(VX, 2)
    XCOLS = 512 * ((VX + 1) // 2)

    def r(ap):
        return ap.bitcast(FPR) if use_f32r else ap

    nc = bacc.Bacc("TRN2", target_bir_lowering=False, debug=False,
                   num_devices=N_CORES)

    XTd = nc.dram_tensor("XT", [XROWS, XCOLS], FP, kind="ExternalInput").ap()
    YTd = nc.dram_tensor("YT", [128, 1024 * PY], FP, kind="ExternalInput").ap()
    ZMd = nc.dram_tensor("ZM", [128, MT * ZP], FP, kind="ExternalInput").ap()
    Cd = nc.dram_tensor("CONSTS", [128, CW], FP, kind="ExternalInput").ap()
    OUTd = nc.dram_tensor("out", [n_sh, T], FP, kind="ExternalOutput").ap()
    OUTr = OUTd.rearrange("(c q p) t -> c p q t", q=4, p=128)

    with tile.TileContext(nc) as tc, ExitStack() as ctx:
        const = ctx.enter_context(tc.tile_pool(name="const", bufs=1))
        big = ctx.enter_context(tc.tile_pool(name="big", bufs=1))
        scr = ctx.enter_context(tc.tile_pool(name="scr", bufs=1))
        actp = ctx.enter_context(tc.tile_pool(name="acts", bufs=4))
        epool = ctx.enter_context(tc.tile_pool(name="ebuf", bufs=5))
        finp = ctx.enter_context(tc.tile_pool(name="fin", bufs=2))

        # ---------------- constants + packed input loads ----------------
        cs = const.tile([128, CW], FP)
        nc.sync.dma_start(r(cs[:]), r(Cd[:]))
        w1 = cs[:, C_W1:C_W1 + H]          # W1 duplicated on rows 64-127
        w2 = cs[:, C_W2:C_W2 + H]
        w3 = cs[:, C_W3:C_W3 + H]
        bs = cs[:, C_BS:C_BS + 3]
        ident = cs[0:ZP, C_ID:C_ID + ZP]
        nh = cs[:, C_NH:C_NH + H]
        onec = cs[:, C_ONE:C_ONE + 1]

        xT = big.tile([XROWS, XCOLS], FP)
        nc.gpsimd.dma_start(r(xT[:]), r(XTd[:]))
        yT = big.tile([128, 1024 * PY], FP)
        for c in range(PY):
            eng = nc.sync if c % 2 == 0 else nc.gpsimd
            eng.dma_start(r(yT[:, 1024 * c:1024 * c + 1024]),
                          r(YTd[:, 1024 * c:1024 * c + 1024]))
        zt = const.tile([128, MT * ZP], FP)
        nc.gpsimd.dma_start(zt[:], ZMd[:])

        def arelu(out, in_, layer, p=128):     # ACT relu
            nc.scalar.activation(out, in_, AF.Relu,
                                 bias=bs[0:p, layer:layer + 1])

        def vrelu(out, in_, layer, p=128):     # DVE relu
            nc.vector.tensor_scalar(out, in_, bs[0:p, layer:layer + 1],
                                    0.0, AluOpType.add, AluOpType.max)

        yf = big.tile([H, m_total], FP)      # Yf^T
        sqy = big.tile([H, m_total], FP)     # Yf^T squared
        xf = big.tile([H, n_sh], FP)         # Xf^T
        sexp = scr.tile([128, MT], FP, tag="sexp")
        zts = const.tile([128, MT * ZP], FP)

        # -------- X MLP (rows 0-31), relus on ACT --------
        xs1 = scr.tile([H, n_sh], FP, tag="xs1")
        xs2 = scr.tile([H, n_sh], FP, tag="xs2")
        with tc.tile_pool(name="xpsum", bufs=2, space="PSUM") as xpp:
            for v in range(VX):
                Rv, cv = (v % 2) * 64, 512 * (v // 2)
                hp = xpp.tile([H, 512], FP, tag="hx")
                nc.tensor.matmul(hp[:], tile_position=(Rv, 0),
                                 lhsT=r(w1[Rv:Rv + D, :]),
                                 rhs=r(xT[Rv:Rv + D, cv:cv + 512]),
                                 start=True, stop=True, skip_group_check=True)
                arelu(r(xs1[:, 512 * v:512 * v + 512]), hp[:], 0, H)
            for v in range(VX):
                hp = xpp.tile([H, 512], FP, tag="hx")
                nc.tensor.matmul(hp[:], tile_position=(0, 0),
                                 lhsT=r(w2[0:H, :]),
                                 rhs=r(xs1[:, 512 * v:512 * v + 512]),
                                 start=True, stop=True, skip_group_check=True)
                arelu(r(xs2[:, 512 * v:512 * v + 512]), hp[:], 1, H)
            for v in range(VX):
                hp = xpp.tile([H, 512], FP, tag="hx")
                nc.tensor.matmul(hp[:], tile_position=(0, 0),
                                 lhsT=r(w3[0:H, :]),
                                 rhs=r(xs2[:, 512 * v:512 * v + 512]),
                                 start=True, stop=True, skip_group_check=True)
                arelu(r(xf[:, 512 * v:512 * v + 512]), hp[:], 2, H)

        # -------- uniform per-chunk pipeline --------
        accs = [None] * IC
        done = [0] * IC

        with (
            tc.tile_pool(name="hpool", bufs=1, space="PSUM") as hpool,
            tc.tile_pool(name="ring", bufs=1, space="PSUM") as ring,
            tc.tile_pool(name="gpool", bufs=2, space="PSUM") as gpool,
            tc.tile_pool(name="accp", bufs=2, space="PSUM") as apool,
        ):
            for ic in range(IC):
                accs[ic] = apool.tile([128, ICW], FP, tag="acc",
                                      name=f"acc{ic}")

            for ch in range(NCH):
                # ---- MLP for Y rows [512ch, 512ch+512) ----
                c, local = ch // 4, ch % 4
                R = 64 * (local // 2)
                col = 1024 * c + 512 * (local % 2)
                hp = hpool.tile([H, 512], FP, tag="hp", name=f"h1_{ch}")
                nc.tensor.matmul(hp[:], tile_position=(R, 0),
                                 lhsT=r(w1[R:R + D, :]),
                                 rhs=r(yT[R:R + D, col:col + 512]),
                                 start=True, stop=True, skip_group_check=True)
                s1 = actp.tile([H, 512], FP, tag="hs")
                vrelu(r(s1[:]), hp[:], 0, H)
                hp = hpool.tile([H, 512], FP, tag="hp", name=f"h2_{ch}")
                nc.tensor.matmul(hp[:], tile_position=(0, 0),
                                 lhsT=r(w2[0:H, :]), rhs=r(s1[:]),
                                 start=True, stop=True, skip_group_check=True)
                s2 = actp.tile([H, 512], FP, tag="hs")
                vrelu(r(s2[:]), hp[:], 1, H)
                hp = hpool.tile([H, 512], FP, tag="hp", name=f"h3_{ch}")
                nc.tensor.matmul(hp[:], tile_position=(0, 0),
                                 lhsT=r(w3[0:H, :]), rhs=r(s2[:]),
                                 start=True, stop=True, skip_group_check=True)
                yfc = yf[:, 512 * ch:512 * ch + 512]
                vrelu(r(yfc), hp[:], 2, H)
                sqc = sqy[:, 512 * ch:512 * ch + 512]
                nc.vector.tensor_mul(r(sqc), yfc, yfc)

                # ---- norm row -> [p=m%128, mt] scatter -> exp -> z-scale ----
                ynp = ring.tile([1, 512], FP, tag="rg", name=f"ynp{ch}")
                nc.tensor.matmul(ynp[:], tile_position=(0, 0),
                                 lhsT=r(nh[0:H, 0:1]), rhs=r(sqc),
                                 start=True, stop=True, skip_group_check=True)
                yns = actp.tile([1, 512], FP, tag="yns")
                nc.vector.tensor_copy(r(yns[:]), ynp[:])
                ntp = ring.tile([128, 4], FP, tag="rg", name=f"ntp{ch}")
                for b in range(4):
                    nc.tensor.matmul(
                        ntp[:, b:b + 1], tile_position=(0, 0),
                        lhsT=yns[0:1, 128 * b:128 * b + 128],
                        rhs=onec[0:1, :], is_transpose=True,
                        start=(b == 0), stop=(b == 3),
                        skip_group_check=True)
                nc.scalar.activation(r(sexp[:, 4 * ch:4 * ch + 4]),
                                     ntp[:], AF.Exp)
                lo = ZP * 4 * ch
                nc.vector.tensor_mul(
                    r(zts[:, lo:lo + 4 * ZP]).rearrange(
                        "p (m z) -> p m z", z=ZP),
                    zt[:, lo:lo + 4 * ZP].rearrange("p (m z) -> p m z", z=ZP),
                    sexp[:, 4 * ch:4 * ch + 4]
                        .rearrange("p m -> p m ()").broadcast_to([128, 4, ZP]))

                # ---- main-loop groups for this chunk's m-tiles ----
                for ic in range(IC):
                    for g0 in range(0, 4, GA):
                        grp = [4 * ch + g0 + t for t in range(GA)]
                        gp = gpool.tile([128, 512 * GA], FP, tag="g")
                        for t, mt in enumerate(grp):
                            nc.tensor.matmul(
                                gp[:, 512 * t:512 * t + 512],
                                tile_position=(0, 0),
                                lhsT=r(yf[0:H, 128 * mt:128 * mt + 128]),
                                rhs=r(xf[0:H, ICW * ic:ICW * ic + ICW]),
                                start=True, stop=True)
                        eb = epool.tile([128, 512 * GA], FP, tag="e")
                        nc.scalar.activation(r(eb[:]), gp[:], AF.Exp)
                        for t, mt in enumerate(grp):
                            nc.tensor.matmul(
                                accs[ic][0:ZP, :],
                                tile_position=(0, 0),
                                lhsT=r(zts[:, ZP * mt:ZP * mt + ZP]),
                                rhs=r(eb[:, 512 * t:512 * t + 512]),
                                start=(done[ic] == 0),
                                stop=(done[ic] == MT - 1),
                                skip_group_check=True)
                            done[ic] += 1

            # ---- normalize + store ----
            for ic in range(IC):
                acc_s = finp.tile([ZP, ICW], FP, tag="accs")
                nc.vector.tensor_copy(acc_s[:], accs[ic][0:ZP, :])
                ot = apool.tile([128, 4 * ZP], FP, tag="acc",
                                name=f"ot{ic}")
                for q in range(4):
                    nc.tensor.matmul(
                        ot[:, ZP * q:ZP * q + ZP],
                        tile_position=(0, 0),
                        lhsT=acc_s[0:ZP, 128 * q:128 * q + 128],
                        rhs=ident,
                        is_transpose=True,
                        start=(q == 0), stop=(q == 3),
                        skip_group_check=True)
                resb = finp.tile([128, 4 * T], FP, tag="res")
                for q in range(4):
                    rec = finp.tile([128, 1], FP, tag="rec")
                    nc.vector.reciprocal(rec[:], ot[:, ZP * q + T:ZP * q + T + 1])
                    nc.vector.tensor_scalar_mul(resb[:, T * q:T * q + T],
                                                ot[:, ZP * q:ZP * q + T],
                                                rec[:])
                nc.sync.dma_start(OUTr[ic],
                                  resb.rearrange("p (q t) -> p q t", q=4))
    nc.compile()
    return nc


def make_in_maps(X, Y, Y_target, W1, b1, W2, b2, W3, b3, n_cores=N_CORES):
    f = lambda a: np.ascontiguousarray(np.asarray(a, dtype=np.float32))
    X, Y, Y_target = f(X), f(Y), f(Y_target)
    W1, W2, W3 = f(W1), f(W2), f(W3)
    b1, b2, b3 = f(b1), f(b2), f(b3)
    m_total = Y.shape[0]
    n_sh = X.shape[0] // n_cores
    MT = m_total // 128
    Z = np.zeros((m_total, ZP), np.float32)
    Z[:, :T] = Y_target
    Z[:, T] = 1.0
    ZM = np.ascontiguousarray(
        Z.reshape(MT, 128, ZP).transpose(1, 0, 2).reshape(128, MT * ZP))
    C = np.zeros((128, CW), np.float32)
    C[:D, C_W1:C_W1 + H] = W1
    C[D:, C_W1:C_W1 + H] = W1
    C[:, C_W2:C_W2 + H] = np.tile(W2, (4, 1))
    C[:, C_W3:C_W3 + H] = np.tile(W3, (4, 1))
    C[:, C_BS:C_BS + 3] = np.stack(
        [np.tile(b1, 4), np.tile(b2, 4), np.tile(b3, 4)], axis=1)
    C[:ZP, C_ID:C_ID + ZP] = np.eye(ZP, dtype=np.float32)
    C[:, C_NH:C_NH + H] = -0.5
    C[:, C_ONE] = 1.0

    def packX(A):  # [d, n]: 512-blocks alternate between two 64-row bands
        d, n = A.shape
        if (n // 512) % 2 != 0:
            return np.ascontiguousarray(A)
        return np.ascontiguousarray(
            A.reshape(d, n // 1024, 2, 512).transpose(2, 0, 1, 3)
             .reshape(2 * d, n // 2))

    def packY(A):  # [d, n]: per 2048-span, 1024-halves stack on row bands
        d, n = A.shape
        assert n % 2048 == 0
        return np.ascontiguousarray(
            A.reshape(d, n // 2048, 2, 1024).transpose(2, 0, 1, 3)
             .reshape(2 * d, n // 2))

    common = dict(YT=packY(Y.T), ZM=ZM, CONSTS=C)
    return [dict(common, XT=packX(X[c * n_sh:(c + 1) * n_sh].T))
            for c in range(n_cores)]


_NC_CACHE = {}


def _get_nc(n_sh, m_total):
    key = (n_sh, m_total)
    if key not in _NC_CACHE:
        use_f32r = os.environ.get("DKR_F32R", "1") == "1"
        _NC_CACHE[key] = build_nc(n_sh, m_total, use_f32r=use_f32r)
    return _NC_CACHE[key]


def kernel(X, Y, Y_target, W1, b1, W2, b2, W3, b3):
    from concourse.bass_utils import run_bass_kernel_spmd

    in_maps = make_in_maps(X, Y, Y_target, W1, b1, W2, b2, W3, b3)
    n_sh = np.asarray(X).shape[0] // N_CORES
    nc = _get_nc(n_sh, np.asarray(Y).shape[0])
    res = run_bass_kernel_spmd(nc, in_maps, core_ids=list(range(N_CORES)))
    return np.concatenate([res.results[c]["out"] for c in range(N_CORES)], axis=0)


# revision 23
# speedup vs baseline: 931.8501x; 1.1401x over previous
"""Trainium2 Bass kernel for DeepKernelRegressionModel.

Math (per core, X sharded by rows across 8 cores):
  Xf = MLP(X), Yf = MLP(Y)                        (3-layer relu MLP, H=32)
  K[i,m] = exp(-|Xf_i - Yf_m|^2 / 2)
  out = (K @ Y_target) / (K @ 1)

Two identities remove all exponent augmentation:
  - the factor exp(-|Xf_i|^2/2) is constant per row i and cancels in the
    normalization, so it is never computed;
  - the factor exp(-|Yf_m|^2/2) is folded into the Z matrix instead of
    the exponent:  z'[m,:] = z[m,:] * exp(-|Yf_m|^2/2).
So mm1 is a pure K=32 product G'[m,i] = Yf_m . Xf_i (m on psum
partitions, 128 per tile), exp runs on ACT, and
  acc[t,i] += z'^T @ exp(G')
accumulates over all m.  A final tiny transpose + reciprocal normalizes.

Hardware constraints that shape the design (verified on neuronx-cc):
  - f32r matmuls (full-rate fp32, 1 cycle/row) are only legal with
    tile_position column 0, so every matmul keeps outputs at partition
    base 0: the MLPs run UNSTACKED on partitions 0-31 and the relus
    (DVE tensor_scalar add+max) pay free-size cost instead;
  - every f32r-matmul operand must be produced through an f32r-typed
    write (loads and relu/square outputs are bitcast);
  - GPSIMD cannot touch PSUM, so Pool only carries DMAs;
  - DMA cost is ~(per-partition bytes x 0.39ns), so X^T/Y^T are host-
    packed two 64-row bands high (W1 is duplicated to rows 64-127).

The kernel runs as one uniform per-chunk pipeline (512 Y rows each):
MLP -> relu (DVE) -> square -> norm matmul [1,512] -> 1-row PE
transposes into [p=m%128, mt] PSUM -> ACT exp -> DVE z-scale, then
immediately the main-loop groups (mm1 -> exp -> mm2) for that chunk's
four m-tiles on both i-chunks, so ACT streams exps almost from the
start and PE never waits on far-away phases.
"""

import os
import numpy as np
from contextlib import ExitStack

import concourse.bass as bass
import concourse.tile as tile
from concourse import bacc, mybir
from concourse.alu_op_type import AluOpType

FP = mybir.dt.float32
FPR = mybir.dt.float32r
AF = mybir.ActivationFunctionType

D, H, T = 64, 32, 8
ZP = 16     # Z columns per m-tile: 8 targets, 1 ones, 7 pad
N_CORES = 8

# packed-constant column layout (consts tile [128, CW])
C_W1, C_W2, C_W3, C_BS, C_ID, C_NH, C_ONE = 0, 32, 64, 96, 99, 115, 147
CW = 160


def build_nc(n_sh, m_total, use_f32r=True, exp_group=2):
    """Build the Bass program for one core (SPMD: same program, all cores).

    n_sh: rows of X handled by this core. m_total: rows of Y (full).
    """
    assert n_sh % 512 == 0 and m_total % 2048 == 0
    MT = m_total // 128       # number of 128-row m-tiles
    NCH = m_total // 512      # number of 512-wide m-chunks
    IC = n_sh // 512          # i-chunks
    ICW = 512
    GA = exp_group            # m-tiles per exp group (4 % GA == 0)
    PY = m_total // 2048      # packed Y loads
    VX = n_sh // 512          # X 512-col chunks
    XROWS = 64 * min(VX, 2)
    XCOLS = 512 * ((VX + 1) // 2)

    def r(ap):
        return ap.bitcast(FPR) if use_f32r else ap

    nc = bacc.Bacc("TRN2", target_bir_lowering=False, debug=False,
                   num_devices=N_CORES)

    XTd = nc.dram_tensor("XT", [XROWS, XCOLS], FP, kind="ExternalInput").ap()
    YTd = nc.dram_tensor("YT", [128, 1024 * PY], FP, kind="ExternalInput").ap()
    ZMd = nc.dram_tensor("ZM", [128, MT * ZP], FP, kind="ExternalInput").ap()
    Cd = nc.dram_tensor("CONSTS", [128, CW], FP, kind="ExternalInput").ap()
    OUTd = nc.dram_tensor("out", [n_sh, T], FP, kind="ExternalOutput").ap()
    OUTr = OUTd.rearrange("(c q p) t -> c p q t", q=4, p=128)

    with tile.TileContext(nc) as tc, ExitStack() as ctx:
        const = ctx.enter_context(tc.tile_pool(name="const", bufs=1))
        big = ctx.enter_context(tc.tile_pool(name="big", bufs=1))
        scr = ctx.enter_context(tc.tile_pool(name="scr", bufs=1))
        actp = ctx.enter_context(tc.tile_pool(name="acts", bufs=4))
        epool = ctx.enter_context(tc.tile_pool(name="ebuf", bufs=5))
        finp = ctx.enter_context(tc.tile_pool(name="fin", bufs=2))

        # ---------------- constants + packed input loads ----------------
        cs = const.tile([128, CW], FP)
        nc.sync.dma_start(r(cs[:]), r(Cd[:]))
        w1 = cs[:, C_W1:C_W1 + H]          # W1 duplicated on rows 64-127
        w2 = cs[:, C_W2:C_W2 + H]
        w3 = cs[:, C_W3:C_W3 + H]
        bs = cs[:, C_BS:C_BS + 3]
        ident = cs[0:ZP, C_ID:C_ID + ZP]
        nh = cs[:, C_NH:C_NH + H]
        onec = cs[:, C_ONE:C_ONE + 1]

        xT = big.tile([XROWS, XCOLS], FP)
        nc.gpsimd.dma_start(r(xT[:]), r(XTd[:]))
        yT = big.tile([128, 1024 * PY], FP)
        for c in range(PY):
            eng = nc.sync if c % 2 == 0 else nc.gpsimd
            eng.dma_start(r(yT[:, 1024 * c:1024 * c + 1024]),
                          r(YTd[:, 1024 * c:1024 * c + 1024]))
        zt = const.tile([128, MT * ZP], FP)
        nc.gpsimd.dma_start(zt[:], ZMd[:])

        def arelu(out, in_, layer, p=128):     # ACT relu
            nc.scalar.activation(out, in_, AF.Relu,
                                 bias=bs[0:p, layer:layer + 1])

        def vrelu(out, in_, layer, p=128):     # DVE relu
            nc.vector.tensor_scalar(out, in_, bs[0:p, layer:layer + 1],
                                    0.0, AluOpType.add, AluOpType.max)

        yf = big.tile([H, m_total], FP)      # Yf^T
        sqy = big.tile([H, m_total], FP)     # Yf^T squared
        xf = big.tile([H, n_sh], FP)         # Xf^T
        sexp = scr.tile([128, MT], FP, tag="sexp")
        zts = const.tile([128, MT * ZP], FP)

        # -------- X MLP (rows 0-31), relus on ACT --------
        xs1 = scr.tile([H, n_sh], FP, tag="xs1")
        xs2 = scr.tile([H, n_sh], FP, tag="xs2")
        with tc.tile_pool(name="xpsum", bufs=2, space="PSUM") as xpp:
            for v in range(VX):
                Rv, cv = (v % 2) * 64, 512 * (v // 2)
                hp = xpp.tile([H, 512], FP, tag="hx")
                nc.tensor.matmul(hp[:], tile_position=(Rv, 0),
                                 lhsT=r(w1[Rv:Rv + D, :]),
                                 rhs=r(xT[Rv:Rv + D, cv:cv + 512]),
                                 start=True, stop=True, skip_group_check=True)
                arelu(r(xs1[:, 512 * v:512 * v + 512]), hp[:], 0, H)
            for v in range(VX):
                hp = xpp.tile([H, 512], FP, tag="hx")
                nc.tensor.matmul(hp[:], tile_position=(0, 0),
                                 lhsT=r(w2[0:H, :]),
                                 rhs=r(xs1[:, 512 * v:512 * v + 512]),
                                 start=True, stop=True, skip_group_check=True)
                arelu(r(xs2[:, 512 * v:512 * v + 512]), hp[:], 1, H)
            for v in range(VX):
                hp = xpp.tile([H, 512], FP, tag="hx")
                nc.tensor.matmul(hp[:], tile_position=(0, 0),
                                 lhsT=r(w3[0:H, :]),
                                 rhs=r(xs2[:, 512 * v:512 * v + 512]),
                                 start=True, stop=True, skip_group_check=True)
                arelu(r(xf[:, 512 * v:512 * v + 512]), hp[:], 2, H)

        # -------- uniform per-chunk pipeline --------
        accs = [None] * IC
        done = [0] * IC

        with (
            tc.tile_pool(name="hpool", bufs=1, space="PSUM") as hpool,
            tc.tile_pool(name="ring", bufs=1, space="PSUM") as ring,
            tc.tile_pool(name="gpool", bufs=2, space="PSUM") as gpool,
            tc.tile_pool(name="accp", bufs=2, space="PSUM") as apool,
        ):
            for ic in range(IC):
                accs[ic] = apool.tile([128, ICW], FP, tag="acc",
                                      name=f"acc{ic}")

            def chain_pre(ch):    # L1 + relu1
                c, local = ch // 4, ch % 4
                R = 64 * (local // 2)
                col = 1024 * c + 512 * (local % 2)
                hp = hpool.tile([H, 512], FP, tag="hp", name=f"h1_{ch}")
                nc.tensor.matmul(hp[:], tile_position=(R, 0),
                                 lhsT=r(w1[R:R + D, :]),
                                 rhs=r(yT[R:R + D, col:col + 512]),
                                 start=True, stop=True, skip_group_check=True)
                s1 = actp.tile([H, 512], FP, tag="hs")
                vrelu(r(s1[:]), hp[:], 0, H)
                return s1

            def chain_mid(ch, s1):  # L2 + relu2
                hp = hpool.tile([H, 512], FP, tag="hp", name=f"h2_{ch}")
                nc.tensor.matmul(hp[:], tile_position=(0, 0),
                                 lhsT=r(w2[0:H, :]), rhs=r(s1[:]),
                                 start=True, stop=True, skip_group_check=True)
                s2 = actp.tile([H, 512], FP, tag="hs")
                vrelu(r(s2[:]), hp[:], 1, H)
                return s2

            def chain_post(ch, s2):  # L3 + relu3 + square
                hp = hpool.tile([H, 512], FP, tag="hp", name=f"h3_{ch}")
                nc.tensor.matmul(hp[:], tile_position=(0, 0),
                                 lhsT=r(w3[0:H, :]), rhs=r(s2[:]),
                                 start=True, stop=True, skip_group_check=True)
                yfc = yf[:, 512 * ch:512 * ch + 512]
                vrelu(r(yfc), hp[:], 2, H)
                sqc = sqy[:, 512 * ch:512 * ch + 512]
                nc.vector.tensor_mul(r(sqc), yfc, yfc)

            def chain_norm(ch):  # norm -> scatter -> exp -> z-scale
                sqc = sqy[:, 512 * ch:512 * ch + 512]
                ynp = ring.tile([1, 512], FP, tag="rg", name=f"ynp{ch}")
                nc.tensor.matmul(ynp[:], tile_position=(0, 0),
                                 lhsT=r(nh[0:H, 0:1]), rhs=r(sqc),
                                 start=True, stop=True, skip_group_check=True)
                yns = actp.tile([1, 512], FP, tag="yns")
                nc.vector.tensor_copy(r(yns[:]), ynp[:])
                ntp = ring.tile([128, 4], FP, tag="rg", name=f"ntp{ch}")
                for b in range(4):
                    nc.tensor.matmul(
                        ntp[:, b:b + 1], tile_position=(0, 0),
                        lhsT=yns[0:1, 128 * b:128 * b + 128],
                        rhs=onec[0:1, :], is_transpose=True,
                        start=(b == 0), stop=(b == 3),
                        skip_group_check=True)
                nc.scalar.activation(r(sexp[:, 4 * ch:4 * ch + 4]),
                                     ntp[:], AF.Exp)
                lo = ZP * 4 * ch
                nc.vector.tensor_mul(
                    r(zts[:, lo:lo + 4 * ZP]).rearrange(
                        "p (m z) -> p m z", z=ZP),
                    zt[:, lo:lo + 4 * ZP].rearrange("p (m z) -> p m z", z=ZP),
                    sexp[:, 4 * ch:4 * ch + 4]
                        .rearrange("p m -> p m ()").broadcast_to([128, 4, ZP]))

            NGC = IC * (4 // GA)   # exp groups per chunk

            def group(ch, j):  # j-th exp group of this chunk
                ic, g0 = j // (4 // GA), (j % (4 // GA)) * GA
                grp = [4 * ch + g0 + t for t in range(GA)]
                gp = gpool.tile([128, 512 * GA], FP, tag="g")
                for t, mt in enumerate(grp):
                    nc.tensor.matmul(
                        gp[:, 512 * t:512 * t + 512],
                        tile_position=(0, 0),
                        lhsT=r(yf[0:H, 128 * mt:128 * mt + 128]),
                        rhs=r(xf[0:H, ICW * ic:ICW * ic + ICW]),
                        start=True, stop=True)
                eb = epool.tile([128, 512 * GA], FP, tag="e")
                nc.scalar.activation(r(eb[:]), gp[:], AF.Exp)
                for t, mt in enumerate(grp):
                    nc.tensor.matmul(
                        accs[ic][0:ZP, :],
                        tile_position=(0, 0),
                        lhsT=r(zts[:, ZP * mt:ZP * mt + ZP]),
                        rhs=r(eb[:, 512 * t:512 * t + 512]),
                        start=(done[ic] == 0),
                        stop=(done[ic] == MT - 1),
                        skip_group_check=True)
                    done[ic] += 1

            # software pipeline: chunk ch+1's chain stages interleave with
            # chunk ch's four exp groups (PE stays in-order but never stalls)
            s1 = chain_pre(0)
            s2 = chain_mid(0, s1)
            chain_post(0, s2)
            chain_norm(0)
            st = {}
            for ch in range(NCH):
                nxt = ch + 1
                for j in range(max(4, NGC)):
                    if nxt < NCH and j < 4:
                        if j == 0:
                            st['s1'] = chain_pre(nxt)
                        elif j == 1:
                            st['s2'] = chain_mid(nxt, st['s1'])
                        elif j == 2:
                            chain_post(nxt, st['s2'])
                        else:
                            chain_norm(nxt)
                    if j < NGC:
                        group(ch, j)

            # ---- normalize + store ----
            for ic in range(IC):
                acc_s = finp.tile([ZP, ICW], FP, tag="accs")
                nc.vector.tensor_copy(acc_s[:], accs[ic][0:ZP, :])
                ot = apool.tile([128, 4 * ZP], FP, tag="acc",
                                name=f"ot{ic}")
                for q in range(4):
                    nc.tensor.matmul(
                        ot[:, ZP * q:ZP * q + ZP],
                        tile_position=(0, 0),
                        lhsT=acc_s[0:ZP, 128 * q:128 * q + 128],
                        rhs=ident,
                        is_transpose=True,
                        start=(q == 0), stop=(q == 3),
                        skip_group_check=True)
                resb = finp.tile([128, 4 * T], FP, tag="res")
                for q in range(4):
                    rec = finp.tile([128, 1], FP, tag="rec")
                    nc.vector.reciprocal(rec[:], ot[:, ZP * q + T:ZP * q + T + 1])
                    nc.vector.tensor_scalar_mul(resb[:, T * q:T * q + T],
                                                ot[:, ZP * q:ZP * q + T],
                                                rec[:])
                nc.sync.dma_start(OUTr[ic],
                                  resb.rearrange("p (q t) -> p q t", q=4))
    nc.compile()
    return nc


def make_in_maps(X, Y, Y_target, W1, b1, W2, b2, W3, b3, n_cores=N_CORES):
    f = lambda a: np.ascontiguousarray(np.asarray(a, dtype=np.float32))
    X, Y, Y_target = f(X), f(Y), f(Y_target)
    W1, W2, W3 = f(W1), f(W2), f(W3)
    b1, b2, b3 = f(b1), f(b2), f(b3)
    m_total = Y.shape[0]
    n_sh = X.shape[0] // n_cores
    MT = m_total // 128
    Z = np.zeros((m_total, ZP), np.float32)
    Z[:, :T] = Y_target
    Z[:, T] = 1.0
    ZM = np.ascontiguousarray(
        Z.reshape(MT, 128, ZP).transpose(1, 0, 2).reshape(128, MT * ZP))
    C = np.zeros((128, CW), np.float32)
    C[:D, C_W1:C_W1 + H] = W1
    C[D:, C_W1:C_W1 + H] = W1
    C[:, C_W2:C_W2 + H] = np.tile(W2, (4, 1))
    C[:, C_W3:C_W3 + H] = np.tile(W3, (4, 1))
    C[:, C_BS:C_BS + 3] = np.stack(
        [np.tile(b1, 4), np.tile(b2, 4), np.tile(b3, 4)], axis=1)
    C[:ZP, C_ID:C_ID + ZP] = np.eye(ZP, dtype=np.float32)
    C[:, C_NH:C_NH + H] = -0.5
    C[:, C_ONE] = 1.0

    def packX(A):  # [d, n]: 512-blocks alternate between two 64-row bands
        d, n = A.shape
        if (n // 512) % 2 != 0:
            return np.ascontiguousarray(A)
        return np.ascontiguousarray(
            A.reshape(d, n // 1024, 2, 512).transpose(2, 0, 1, 3)
             .reshape(2 * d, n // 2))

    def packY(A):  # [d, n]: per 2048-span, 1024-halves stack on row bands
        d, n = A.shape
        assert n % 2048 == 0
        return np.ascontiguousarray(
            A.reshape(d, n // 2048, 2, 1024).transpose(2, 0, 1, 3)
             .reshape(2 * d, n // 2))

    common = dict(YT=packY(Y.T), ZM=ZM, CONSTS=C)
    return [dict(common, XT=packX(X[c * n_sh:(c + 1) * n_sh].T))
            for c in range(n_cores)]


_NC_CACHE = {}


def _get_nc(n_sh, m_total):
    key = (n_sh, m_total)
    if key not in _NC_CACHE:
        use_f32r = os.environ.get("DKR_F32R", "1") == "1"
        _NC_CACHE[key] = build_nc(n_sh, m_total, use_f32r=use_f32r)
    return _NC_CACHE[key]


def kernel(X, Y, Y_target, W1, b1, W2, b2, W3, b3):
    from concourse.bass_utils import run_bass_kernel_spmd

    in_maps = make_in_maps(X, Y, Y_target, W1, b1, W2, b2, W3, b3)
    n_sh = np.asarray(X).shape[0] // N_CORES
    nc = _get_nc(n_sh, np.asarray(Y).shape[0])
    res = run_bass_kernel_spmd(nc, in_maps, core_ids=list(range(N_CORES)))
    return np.concatenate([res.results[c]["out"] for c in range(N_CORES)], axis=0)
